# revision 1
# baseline (speedup 1.0000x reference)
"""nn_CrossAttention Trainium2 kernel — 8-core data-parallel over batch.

Per core (batch slice b=1):
  q1^T / kv1^T via transposed-orientation 1x1 convs (stationary = input rows,
  f32r matmuls), depthwise 3x3/7x7 as per-(channel, dh) banded-Toeplitz
  matmuls on the TensorEngine (host-built bf16 Toeplitz tiles, PSUM
  accumulation over dh with free-dim h shifts), l2-norm folded into attention
  scales, QK^T with n on partitions, softmax without max-subtraction
  (|logits| <= temperature), attn@v merged with the output 1x1 conv through
  a per-pair [96,192] fused matrix.
"""

import sys

sys.path.insert(0, "/opt/trn_rl_repo")

import numpy as np
import ml_dtypes

B, C, Himg, Wimg = 8, 192, 128, 128
HW = Himg * Wimg
HEADS, DHC = 4, 48      # heads, channels per head
PC = 96                 # channels per head-pair
NW = 8                  # Toeplitz tiles per DMA wave
SLAB = 8                # image rows per input stream slab

_PROG = None            # cached (nc, meta)


def _build_toeplitz(wdw, ksz):
    """wdw [c, ksz, ksz] f32 -> [c*ksz, 128, 128] bf16, index = c*ksz + dh.

    T[w_in, w_out] = wdw[c, dh, w_in - w_out + pad] inside the band, else 0.
    """
    pad = ksz // 2
    wi = np.arange(128)[:, None]
    wo = np.arange(128)[None, :]
    idx = wi - wo + pad
    valid = (idx >= 0) & (idx < ksz)
    idxc = np.clip(idx, 0, ksz - 1)
    T = wdw[:, :, idxc] * valid[None, None]
    return np.ascontiguousarray(T.reshape(-1, 128, 128).astype(ml_dtypes.bfloat16))


def _split_excess_waits(nc, limit=1):
    """This container's walrus rejects >1 sync wait per instruction (and any
    wait on Drain beyond its own barrier). Hoist extras onto same-engine
    NoOps placed immediately before."""
    import bass_rust
    import concourse.mybir as mybir

    n_split = 0
    for fn in nc.m.functions:
        for bb in fn.blocks:
            insts = bb.instructions
            i = 0
            while i < len(insts):
                inst = insts[i]
                si = inst.sync_info
                lim = 0 if type(inst).__name__ == "InstDrain" else limit
                if si is not None and si.on_wait and len(si.on_wait) > lim:
                    waits = list(si.on_wait)
                    keep, extra = waits[:lim], waits[lim:]
                    pos = i
                    for j in range(0, len(extra), max(limit, 1)):
                        ch = extra[j : j + max(limit, 1)]
                        nop = mybir.InstNoOp(
                            name=f"waitsplit_{n_split}_{pos}",
                            engine=inst.engine,
                            ins=[],
                            outs=[],
                            sync_info=bass_rust.SyncInfo(on_wait=ch, on_update=[]),
                        )
                        insts.insert(pos, nop)
                        pos += 1
                        n_split += 1
                    inst.sync_info = bass_rust.SyncInfo(
                        on_wait=keep, on_update=list(si.on_update)
                    )
                    i = pos + 1
                else:
                    i += 1
    return n_split


def _build_program():
    import concourse.bass as bass
    import concourse.mybir as mybir
    import concourse.tile as tile

    F32 = mybir.dt.float32
    F32R = mybir.dt.float32r
    BF16 = mybir.dt.bfloat16
    AF = mybir.ActivationFunctionType
    OP = mybir.AluOpType

    nc = bass.Bass("TRN2", target_bir_lowering=False, debug=False, num_devices=8)

    # ---- DRAM parameters ----
    xin = nc.dram_tensor("x", [C, HW], F32R, kind="ExternalInput").ap()
    yin = nc.dram_tensor("y", [C, HW], F32R, kind="ExternalInput").ap()
    wq_d = nc.dram_tensor("wq", [C, 256], F32R, kind="ExternalInput").ap()
    wkv_d = nc.dram_tensor("wkv", [C, 384], F32R, kind="ExternalInput").ap()
    wp_d = nc.dram_tensor("wp", [2, PC, C], BF16, kind="ExternalInput").ap()
    tq_d = nc.dram_tensor("tq", [C * 3, 128, 128], BF16, kind="ExternalInput").ap()
    tkv_d = nc.dram_tensor("tkv", [2 * C * 7, 128, 128], BF16, kind="ExternalInput").ap()
    idb_d = nc.dram_tensor("idb", [128, 128], BF16, kind="ExternalInput").ap()
    mask_d = nc.dram_tensor("maskbd", [PC, PC], F32, kind="ExternalInput").ap()
    temp_d = nc.dram_tensor("temprow", [1, C], F32, kind="ExternalInput").ap()
    out_d = nc.dram_tensor("out", [C, HW], F32, kind="ExternalOutput").ap()

    with tile.TileContext(nc) as tc:
        import contextlib

        with contextlib.ExitStack() as ctx:
            consts = ctx.enter_context(tc.tile_pool(name="consts", bufs=1))
            s1 = ctx.enter_context(tc.tile_pool(name="s1", bufs=1))
            s2 = ctx.enter_context(tc.tile_pool(name="s2", bufs=1))
            s3 = ctx.enter_context(tc.tile_pool(name="s3", bufs=1))
            streams = ctx.enter_context(tc.tile_pool(name="streams", bufs=2))
            tpool = ctx.enter_context(tc.tile_pool(name="tpool", bufs=2))
            ps = ctx.enter_context(tc.tile_pool(name="ps", bufs=4, space="PSUM"))
            pst = ctx.enter_context(tc.tile_pool(name="pst", bufs=2, space="PSUM"))
            scratch = ctx.enter_context(tc.tile_pool(name="scratch", bufs=2))
            ostage = ctx.enter_context(tc.tile_pool(name="ostage", bufs=2))
            misc = ctx.enter_context(tc.tile_pool(name="misc", bufs=4))
            attnp_pool = ctx.enter_context(tc.tile_pool(name="attnp", bufs=2, space="PSUM"))
            stats = ctx.enter_context(tc.tile_pool(name="stats", bufs=1))

            # ---- load constants ----
            wq0 = consts.tile([128, 256], F32R)
            wq1 = consts.tile([64, 256], F32R)
            nc.sync.dma_start(out=wq0, in_=wq_d[0:128, :])
            nc.sync.dma_start(out=wq1, in_=wq_d[128:192, :])
            wkv0 = consts.tile([128, 384], F32R)
            wkv1 = consts.tile([64, 384], F32R)
            nc.sync.dma_start(out=wkv0, in_=wkv_d[0:128, :])
            nc.sync.dma_start(out=wkv1, in_=wkv_d[128:192, :])
            wp0 = consts.tile([PC, C], BF16)
            wp1 = consts.tile([PC, C], BF16)
            nc.sync.dma_start(out=wp0, in_=wp_d[0])
            nc.sync.dma_start(out=wp1, in_=wp_d[1])
            identb = consts.tile([128, 128], BF16)
            nc.sync.dma_start(out=identb, in_=idb_d)
            maskbd = consts.tile([PC, PC], F32)
            nc.sync.dma_start(out=maskbd, in_=mask_d)
            temprow = consts.tile([1, C], F32)
            nc.sync.dma_start(out=temprow, in_=temp_d)
            onescol = consts.tile([128, 1], BF16)
            nc.vector.memset(onescol, 1.0)
            ones1 = consts.tile([1, 128], BF16)
            nc.vector.memset(ones1, 1.0)

            # ---- big SBUF regions ----
            # B_q / B_k layouts: [w partitions, h*192 + c], bf16 (B_k padded +32)
            bq = s1.tile([128, Himg * C], BF16, tag="qv")
            bk = s2.tile([128, Himg * C + 32], BF16, tag="kk")
            bv = s3.tile([128, Himg * C], BF16, tag="vv")
            bq3 = bq.rearrange("p (h c) -> p h c", c=C)
            bk3 = bk[:, 0 : Himg * C].rearrange("p (h c) -> p h c", c=C)
            bv3 = bv.rearrange("p (h c) -> p h c", c=C)

            def chan_ap(region3, c, col0, cnt):
                # [128, cnt] strided view: channel c, h-columns col0..col0+cnt
                return region3[:, col0 : col0 + cnt, c]

            # ================= Phase A: 1x1 convs (transposed orientation) ==
            def conv1x1_phase(src_d, mov0, mov1, nmov, wb):
                for h in range(Himg):
                    sl = h % SLAB
                    if sl == 0:
                        xs0 = streams.tile([128, SLAB * 128], F32R, tag="st0")
                        xs1 = streams.tile([64, SLAB * 128], F32R, tag="st1")
                        nc.sync.dma_start(
                            out=xs0, in_=src_d[0:128, h * 128 : (h + SLAB) * 128]
                        )
                        nc.sync.dma_start(
                            out=xs1, in_=src_d[128:192, h * 128 : (h + SLAB) * 128]
                        )
                    pt = ps.tile([128, nmov], F32, tag="ps")
                    nc.tensor.matmul(
                        pt, xs0[:, sl * 128 : (sl + 1) * 128], mov0,
                        start=True, stop=False,
                    )
                    nc.tensor.matmul(
                        pt, xs1[:, sl * 128 : (sl + 1) * 128], mov1,
                        start=False, stop=True,
                    )
                    wb(h, pt)

            def wb_q(h, pt):
                nc.vector.tensor_copy(bq[:, h * C : (h + 1) * C], pt[:, 0:C])

            def wb_kv(h, pt):
                nc.vector.tensor_copy(bk[:, h * C : (h + 1) * C], pt[:, 0:C])
                nc.vector.tensor_copy(bv[:, h * C : (h + 1) * C], pt[:, C : 2 * C])

            conv1x1_phase(xin, wq0, wq1, 256, wb_q)
            conv1x1_phase(yin, wkv0, wkv1, 384, wb_kv)

            # ================= Phase B: depthwise via Toeplitz matmuls ======
            def dw_phase(region3, t_dram, ksz, ch0):
                pad = ksz // 2
                order = [pad] + [d for d in range(ksz) if d != pad]
                cw = max(1, NW // ksz)      # channels per T-wave
                wave_tile = [None]
                for ci in range(C):
                    if ci % cw == 0:
                        nt = min(cw, C - ci) * ksz
                        wave_tile[0] = tpool.tile([128, cw * ksz, 128], BF16, tag="tw", name=f"tw_{ch0}_{ci}")
                        i0 = (ch0 + ci) * ksz
                        nc.sync.dma_start(
                            out=wave_tile[0][:, 0:nt, :],
                            in_=t_dram[i0 : i0 + nt].rearrange("i p c -> p i c"),
                        )
                    tw = wave_tile[0]
                    pdw = ps.tile([128, 128], F32, tag="ps")
                    base = (ci % cw) * ksz
                    for j, dh in enumerate(order):
                        sh = dh - pad
                        cnt = Himg - abs(sh)
                        h0o, h0i = max(0, -sh), max(0, sh)
                        nc.tensor.matmul(
                            pdw[:, h0o : h0o + cnt],
                            tw[:, base + dh, :],
                            chan_ap(region3, ci, h0i, cnt),
                            start=(j == 0),
                            stop=(j == len(order) - 1),
                        )
                    nc.vector.tensor_copy(chan_ap(region3, ci, 0, Himg), pdw)

            dw_phase(bq3, tq_d, 3, 0)
            dw_phase(bk3, tkv_d, 7, 0)

            # ================= Phase C: sum-of-squares for l2 norms =========
            partials = stats.tile([128, 2 * C], F32)
            partials_bf = stats.tile([128, 2 * C], BF16)
            for t, region3 in enumerate((bq3, bk3)):
                for ci in range(C):
                    sc = scratch.tile([128, 128], BF16, tag="sq")
                    nc.scalar.activation(
                        out=sc,
                        in_=chan_ap(region3, ci, 0, Himg),
                        func=AF.Square,
                        accum_out=partials[:, t * C + ci : t * C + ci + 1],
                    )
            nc.vector.tensor_copy(partials_bf, partials)

            # ================= Phase D: QK^T + softmax prep per pair ========
            ezs = []
            for P in range(2):
                attnp = attnp_pool.tile([128, PC], F32, tag="at")
                for h in range(Himg):
                    nc.tensor.matmul(
                        attnp,
                        bk[:, h * C + PC * P : h * C + PC * P + 128],
                        bq[:, h * C + PC * P : h * C + PC * P + PC],
                        start=(h == 0),
                        stop=(h == Himg - 1),
                    )
                # rq as a row [1, PC]: colsum of q-partials then 1/sqrt, * temp
                prow = ps.tile([1, PC], F32, tag="ps")
                nc.tensor.matmul(
                    prow, onescol, partials_bf[:, PC * P : PC * P + PC],
                    start=True, stop=True,
                )
                sq_row = misc.tile([1, PC], F32, tag="m1")
                nc.scalar.activation(out=sq_row, in_=prow, func=AF.Sqrt)
                rq_row = misc.tile([1, PC], F32, tag="m2")
                nc.vector.reciprocal(rq_row, sq_row)
                nc.vector.tensor_tensor(
                    rq_row, rq_row, temprow[:, PC * P : PC * P + PC], op=OP.mult
                )
                rq_bf = misc.tile([1, PC], BF16, tag="m3")
                nc.vector.tensor_copy(rq_bf, rq_row)
                # rk as a column [PC, 1]
                pcol = ps.tile([PC, 1], F32, tag="ps")
                nc.tensor.matmul(
                    pcol, partials_bf[:, C + PC * P : C + PC * P + PC], onescol,
                    start=True, stop=True,
                )
                sq_col = misc.tile([PC, 1], F32, tag="m4")
                nc.scalar.activation(out=sq_col, in_=pcol, func=AF.Sqrt)
                rk_col = misc.tile([PC, 1], F32, tag="m5")
                nc.vector.reciprocal(rk_col, sq_col)
                # rq replicated across partitions via K=1 matmul
                prep = ps.tile([PC, PC], F32, tag="ps")
                nc.tensor.matmul(
                    prep, ones1[:, 0:PC], rq_bf, start=True, stop=True
                )
                rqrep = misc.tile([PC, PC], F32, tag="m6")
                nc.vector.tensor_copy(rqrep, prep)
                t1 = misc.tile([PC, PC], F32, tag="m7")
                nc.vector.tensor_tensor(t1, attnp[0:PC, :], rqrep, op=OP.mult)
                # exp(rk * t1), then zero junk blocks, bf16
                e1 = misc.tile([PC, PC], F32, tag="m8")
                nc.scalar.activation(out=e1, in_=t1, func=AF.Exp, scale=rk_col)
                ezero = stats.tile([PC, 128], BF16, tag=f"ez{P}")
                nc.vector.memset(ezero[:, PC:128], 0.0)
                nc.vector.tensor_tensor(ezero[:, 0:PC], e1, maskbd, op=OP.mult)
                # column sums -> recip
                pcs = ps.tile([PC, 1], F32, tag="ps")
                nc.tensor.matmul(
                    pcs, ezero[:, 0:PC], onescol[0:PC], start=True, stop=True
                )
                recip = stats.tile([PC, 1], F32, tag=f"rc{P}")
                nc.vector.reciprocal(recip, pcs)
                ezs.append((ezero, recip))

            # ================= Phase E: v depthwise =========================
            dw_phase(bv3, tkv_d, 7, C)

            # ================= Phase F: transpose v -> [c, hw] ==============
            vt = s1.tile([PC, 2 * HW], BF16, tag="qv")
            for P in range(2):
                for h in range(Himg):
                    ptv = pst.tile([PC, 128], BF16, tag="tp")
                    nc.tensor.transpose(
                        ptv, bv[:, h * C + PC * P : h * C + PC * P + PC], identb
                    )
                    nc.vector.tensor_copy(
                        vt[:, P * HW + h * 128 : P * HW + (h + 1) * 128], ptv
                    )

            # ================= Phase G: fused (attn @ v) + proj =============
            mps = []
            for P in range(2):
                ezero, recip = ezs[P]
                ezt_ps = pst.tile([PC, PC], BF16, tag="tp")
                nc.tensor.transpose(ezt_ps, ezero[:, 0:PC], identb[0:PC, 0:PC])
                ezt = misc.tile([PC, PC], BF16, tag="m9")
                nc.vector.tensor_copy(ezt, ezt_ps)
                wsc = misc.tile([PC, C], BF16, tag="m10")
                nc.vector.tensor_scalar_mul(wsc, (wp0, wp1)[P], recip)
                pmp = ps.tile([PC, C], F32, tag="ps")
                nc.tensor.matmul(pmp, ezt, wsc, start=True, stop=True)
                mp = stats.tile([PC, C], BF16, tag=f"mp{P}")
                nc.vector.tensor_copy(mp, pmp)
                mps.append(mp)

            for mi, (r0, r1) in enumerate(((0, 128), (128, 192))):
                mw = r1 - r0
                for n in range(0, HW, 512):
                    po = ps.tile([mw, 512], F32, tag="ps")
                    nc.tensor.matmul(
                        po, mps[0][:, r0:r1], vt[:, n : n + 512],
                        start=True, stop=False,
                    )
                    nc.tensor.matmul(
                        po, mps[1][:, r0:r1], vt[:, HW + n : HW + n + 512],
                        start=False, stop=True,
                    )
                    so = ostage.tile([mw, 512], F32, tag="os")
                    nc.vector.tensor_copy(so, po)
                    nc.sync.dma_start(out=out_d[r0:r1, n : n + 512], in_=so)

    _split_excess_waits(nc)
    return nc


def _get_program():
    global _PROG
    if _PROG is None:
        _PROG = _build_program()
    return _PROG


def kernel(x, y, q_w, q_dw_w, kv_w, kv_dw_w, proj_w, temperature):
    return _run(x, y, q_w, q_dw_w, kv_w, kv_dw_w, proj_w, temperature)[0]


def _run(x, y, q_w, q_dw_w, kv_w, kv_dw_w, proj_w, temperature, trace=False):
    from concourse.bass_utils import run_bass_kernel_spmd

    x = np.asarray(x, dtype=np.float32).reshape(B, C, HW)
    y = np.asarray(y, dtype=np.float32).reshape(B, C, HW)
    q_w = np.asarray(q_w, dtype=np.float32)
    kv_w = np.asarray(kv_w, dtype=np.float32)
    proj_w = np.asarray(proj_w, dtype=np.float32)
    q_dw_w = np.asarray(q_dw_w, dtype=np.float32)
    kv_dw_w = np.asarray(kv_dw_w, dtype=np.float32)
    temperature = np.asarray(temperature, dtype=np.float32).reshape(HEADS)

    wq = np.zeros((C, 256), np.float32)
    wq[:, 0:C] = q_w[:, :, 0, 0].T
    wkv = np.ascontiguousarray(kv_w[:, :, 0, 0].T)          # [C, 2C]
    wpT = proj_w[:, :, 0, 0].T                              # [c_in, c_out]
    wp = np.stack([wpT[0:PC], wpT[PC:C]]).astype(ml_dtypes.bfloat16)
    tq = _build_toeplitz(q_dw_w[:, 0], 3)
    tkv = _build_toeplitz(kv_dw_w[:, 0], 7)
    idb = np.eye(128, dtype=ml_dtypes.bfloat16)
    maskbd = np.zeros((PC, PC), np.float32)
    maskbd[0:DHC, 0:DHC] = 1.0
    maskbd[DHC:PC, DHC:PC] = 1.0
    temprow = np.repeat(temperature, DHC).reshape(1, C)

    shared = {
        "wq": wq, "wkv": wkv, "wp": wp, "tq": tq, "tkv": tkv,
        "idb": idb, "maskbd": maskbd, "temprow": temprow,
    }
    in_maps = [dict(shared, x=x[i], y=y[i]) for i in range(B)]

    nc = _get_program()
    res = run_bass_kernel_spmd(
        nc, in_maps, core_ids=list(range(B)), trace=trace
    )
    out = np.stack([res.results[i]["out"] for i in range(B)])
    return out.reshape(B, C, Himg, Wimg).astype(np.float32), res



# revision 10
# speedup vs baseline: 1.1961x; 1.1961x over previous
"""nn_CrossAttention Trainium2 kernel — 8-core data-parallel over batch.

Per core (batch slice b=1):
  q1^T / kv1^T via transposed-orientation 1x1 convs (stationary = input rows,
  f32r matmuls), depthwise 3x3/7x7 as per-(channel, dh) banded-Toeplitz
  matmuls on the TensorEngine (host-built bf16 Toeplitz tiles, PSUM
  accumulation over dh with free-dim h shifts), l2-norm folded into attention
  scales, QK^T with n on partitions, softmax without max-subtraction
  (|logits| <= temperature), attn@v merged with the output 1x1 conv through
  a per-pair [96,192] fused matrix.
"""

import sys

sys.path.insert(0, "/opt/trn_rl_repo")

import numpy as np
import ml_dtypes

B, C, Himg, Wimg = 8, 192, 128, 128
HW = Himg * Wimg
HEADS, DHC = 4, 48      # heads, channels per head
PC = 96                 # channels per head-pair
NW = 8                  # Toeplitz tiles per DMA wave
SLAB = 8                # image rows per input stream slab

_PROG = None            # cached (nc, meta)


def _build_toeplitz(wdw, ksz):
    """wdw [c, ksz, ksz] f32 -> [128, c*ksz, 128] bf16, tile index = c*ksz + dh.

    T[w_in, tile, w_out] = wdw[c, dh, w_in - w_out + pad] inside the band,
    else 0.  Partition-major so a wave DMA reads contiguous bytes per
    partition.
    """
    pad = ksz // 2
    wi = np.arange(128)[:, None]
    wo = np.arange(128)[None, :]
    idx = wi - wo + pad
    valid = (idx >= 0) & (idx < ksz)
    idxc = np.clip(idx, 0, ksz - 1)
    T = wdw[:, :, idxc] * valid[None, None]          # [c, ksz, 128, 128]
    T = T.reshape(-1, 128, 128).transpose(1, 0, 2)   # [128, c*ksz, 128]
    return np.ascontiguousarray(T.astype(ml_dtypes.bfloat16))


def _split_excess_waits(nc, limit=1):
    """This container's walrus rejects >1 sync wait per instruction (and any
    wait on Drain beyond its own barrier). Hoist extras onto same-engine
    NoOps placed immediately before."""
    import bass_rust
    import concourse.mybir as mybir

    n_split = 0
    for fn in nc.m.functions:
        for bb in fn.blocks:
            insts = bb.instructions
            i = 0
            while i < len(insts):
                inst = insts[i]
                si = inst.sync_info
                lim = 0 if type(inst).__name__ == "InstDrain" else limit
                if si is not None and si.on_wait and len(si.on_wait) > lim:
                    waits = list(si.on_wait)
                    keep, extra = waits[:lim], waits[lim:]
                    pos = i
                    for j in range(0, len(extra), max(limit, 1)):
                        ch = extra[j : j + max(limit, 1)]
                        nop = mybir.InstNoOp(
                            name=f"waitsplit_{n_split}_{pos}",
                            engine=inst.engine,
                            ins=[],
                            outs=[],
                            sync_info=bass_rust.SyncInfo(on_wait=ch, on_update=[]),
                        )
                        insts.insert(pos, nop)
                        pos += 1
                        n_split += 1
                    inst.sync_info = bass_rust.SyncInfo(
                        on_wait=keep, on_update=list(si.on_update)
                    )
                    i = pos + 1
                else:
                    i += 1
    return n_split


def _build_program():
    import concourse.bass as bass
    import concourse.mybir as mybir
    import concourse.tile as tile

    F32 = mybir.dt.float32
    F32R = mybir.dt.float32r
    BF16 = mybir.dt.bfloat16
    AF = mybir.ActivationFunctionType
    OP = mybir.AluOpType

    nc = bass.Bass("TRN2", target_bir_lowering=False, debug=False, num_devices=8)

    # ---- DRAM parameters ----
    xin = nc.dram_tensor("x", [C, HW], BF16, kind="ExternalInput").ap()
    yin = nc.dram_tensor("y", [C, HW], BF16, kind="ExternalInput").ap()
    wq_d = nc.dram_tensor("wq", [C, C], BF16, kind="ExternalInput").ap()
    wkv_d = nc.dram_tensor("wkv", [C, 384], BF16, kind="ExternalInput").ap()
    wp_d = nc.dram_tensor("wp", [2, PC, C], BF16, kind="ExternalInput").ap()
    tq_d = nc.dram_tensor("tq", [128, C * 3, 128], BF16, kind="ExternalInput").ap()
    tkv_d = nc.dram_tensor("tkv", [128, 2 * C * 7, 128], BF16, kind="ExternalInput").ap()
    idb_d = nc.dram_tensor("idb", [128, 128], BF16, kind="ExternalInput").ap()
    mask_d = nc.dram_tensor("maskbd", [PC, PC], F32, kind="ExternalInput").ap()
    temp_d = nc.dram_tensor("temprow", [1, C], F32, kind="ExternalInput").ap()
    out_d = nc.dram_tensor("out", [C, HW], F32, kind="ExternalOutput").ap()

    with tile.TileContext(nc) as tc:
        import contextlib

        with contextlib.ExitStack() as ctx:
            consts = ctx.enter_context(tc.tile_pool(name="consts", bufs=1))
            s1 = ctx.enter_context(tc.tile_pool(name="s1", bufs=1))
            s2 = ctx.enter_context(tc.tile_pool(name="s2", bufs=1))
            s3 = ctx.enter_context(tc.tile_pool(name="s3", bufs=1))
            streams = ctx.enter_context(tc.tile_pool(name="streams", bufs=2))
            tpool = ctx.enter_context(tc.tile_pool(name="tpool", bufs=2))
            ps = ctx.enter_context(tc.tile_pool(name="ps", bufs=4, space="PSUM"))
            pst = ctx.enter_context(tc.tile_pool(name="pst", bufs=2, space="PSUM"))
            scratch = ctx.enter_context(tc.tile_pool(name="scratch", bufs=2))
            ostage = ctx.enter_context(tc.tile_pool(name="ostage", bufs=2))
            misc = ctx.enter_context(tc.tile_pool(name="misc", bufs=4))
            attnp_pool = ctx.enter_context(tc.tile_pool(name="attnp", bufs=2, space="PSUM"))
            stats = ctx.enter_context(tc.tile_pool(name="stats", bufs=1))

            # ---- load constants ----
            wq0 = consts.tile([128, C], BF16)
            wq1 = consts.tile([64, C], BF16)
            nc.sync.dma_start(out=wq0, in_=wq_d[0:128, :])
            nc.sync.dma_start(out=wq1, in_=wq_d[128:192, :])
            wkv0 = consts.tile([128, 384], BF16)
            wkv1 = consts.tile([64, 384], BF16)
            nc.sync.dma_start(out=wkv0, in_=wkv_d[0:128, :])
            nc.sync.dma_start(out=wkv1, in_=wkv_d[128:192, :])
            wp0 = consts.tile([PC, C], BF16)
            wp1 = consts.tile([PC, C], BF16)
            nc.sync.dma_start(out=wp0, in_=wp_d[0])
            nc.sync.dma_start(out=wp1, in_=wp_d[1])
            identb = consts.tile([128, 128], BF16)
            nc.sync.dma_start(out=identb, in_=idb_d)
            maskbd = consts.tile([PC, PC], F32)
            nc.sync.dma_start(out=maskbd, in_=mask_d)
            temprow = consts.tile([1, C], F32)
            nc.sync.dma_start(out=temprow, in_=temp_d)
            onescol = consts.tile([128, 1], BF16)
            nc.vector.memset(onescol, 1.0)
            ones1 = consts.tile([1, 128], BF16)
            nc.vector.memset(ones1, 1.0)

            # ---- big SBUF regions ----
            # B_q / B_k layouts: [w partitions, h*192 + c], bf16 (B_k padded +32)
            bq = s1.tile([128, Himg * C], BF16, tag="qv")
            bk = s2.tile([128, Himg * C + 32], BF16, tag="kk")
            bv = s3.tile([128, Himg * C], BF16, tag="vv")
            bq3 = bq.rearrange("p (h c) -> p h c", c=C)
            bk3 = bk[:, 0 : Himg * C].rearrange("p (h c) -> p h c", c=C)
            bv3 = bv.rearrange("p (h c) -> p h c", c=C)

            def chan_ap(region3, c, col0, cnt):
                # [128, cnt] strided view: channel c, h-columns col0..col0+cnt
                return region3[:, col0 : col0 + cnt, c]

            # ================= Phase A: 1x1 convs (transposed orientation) ==
            def conv1x1_phase(src_d, mov0, mov1, nmov, wb):
                for h in range(Himg):
                    sl = h % SLAB
                    if sl == 0:
                        xs0 = streams.tile([128, SLAB * 128], BF16, tag="st0")
                        xs1 = streams.tile([64, SLAB * 128], BF16, tag="st1")
                        nc.sync.dma_start(
                            out=xs0, in_=src_d[0:128, h * 128 : (h + SLAB) * 128]
                        )
                        nc.sync.dma_start(
                            out=xs1, in_=src_d[128:192, h * 128 : (h + SLAB) * 128]
                        )
                    pt = ps.tile([128, nmov], F32, tag="ps")
                    nc.tensor.matmul(
                        pt, xs0[:, sl * 128 : (sl + 1) * 128], mov0,
                        start=True, stop=False,
                    )
                    nc.tensor.matmul(
                        pt, xs1[:, sl * 128 : (sl + 1) * 128], mov1,
                        start=False, stop=True,
                    )
                    wb(h, pt)

            def wb_q(h, pt):
                nc.vector.tensor_copy(bq[:, h * C : (h + 1) * C], pt[:, 0:C])

            def wb_kv(h, pt):
                nc.vector.tensor_copy(bk[:, h * C : (h + 1) * C], pt[:, 0:C])
                nc.vector.tensor_copy(bv[:, h * C : (h + 1) * C], pt[:, C : 2 * C])

            conv1x1_phase(xin, wq0, wq1, C, wb_q)
            conv1x1_phase(yin, wkv0, wkv1, 384, wb_kv)

            # ================= Phase B: depthwise via Toeplitz matmuls ======
            def dw_phase(region3, t_dram, ksz, ch0):
                pad = ksz // 2
                order = [pad] + [d for d in range(ksz) if d != pad]
                cw = max(1, NW // ksz)      # channels per T-wave
                wave_tile = [None]
                for ci in range(C):
                    if ci % cw == 0:
                        nt = min(cw, C - ci) * ksz
                        wave_tile[0] = tpool.tile([128, cw * ksz, 128], BF16, tag="tw", name=f"tw_{ch0}_{ci}")
                        i0 = (ch0 + ci) * ksz
                        nc.sync.dma_start(
                            out=wave_tile[0][:, 0:nt, :],
                            in_=t_dram[:, i0 : i0 + nt, :],
                        )
                    tw = wave_tile[0]
                    pdw = ps.tile([128, 128], F32, tag="ps")
                    base = (ci % cw) * ksz
                    for j, dh in enumerate(order):
                        sh = dh - pad
                        cnt = Himg - abs(sh)
                        h0o, h0i = max(0, -sh), max(0, sh)
                        nc.tensor.matmul(
                            pdw[:, h0o : h0o + cnt],
                            tw[:, base + dh, :],
                            chan_ap(region3, ci, h0i, cnt),
                            start=(j == 0),
                            stop=(j == len(order) - 1),
                        )
                    nc.vector.tensor_copy(chan_ap(region3, ci, 0, Himg), pdw)

            dw_phase(bq3, tq_d, 3, 0)
            dw_phase(bk3, tkv_d, 7, 0)

            # ================= Phase C: sum-of-squares for l2 norms =========
            partials = stats.tile([128, 2 * C], F32)
            partials_bf = stats.tile([128, 2 * C], BF16)
            for t, region3 in enumerate((bq3, bk3)):
                for ci in range(C):
                    sc = scratch.tile([128, 128], BF16, tag="sq")
                    nc.scalar.activation(
                        out=sc,
                        in_=chan_ap(region3, ci, 0, Himg),
                        func=AF.Square,
                        accum_out=partials[:, t * C + ci : t * C + ci + 1],
                    )
            nc.vector.tensor_copy(partials_bf, partials)

            # ================= Phase D: QK^T + softmax prep per pair ========
            ezs = []
            for P in range(2):
                attnp = attnp_pool.tile([128, PC], F32, tag="at")
                for h in range(Himg):
                    nc.tensor.matmul(
                        attnp,
                        bk[:, h * C + PC * P : h * C + PC * P + 128],
                        bq[:, h * C + PC * P : h * C + PC * P + PC],
                        start=(h == 0),
                        stop=(h == Himg - 1),
                    )
                # rq as a row [1, PC]: colsum of q-partials then 1/sqrt, * temp
                prow = ps.tile([1, PC], F32, tag="ps")
                nc.tensor.matmul(
                    prow, onescol, partials_bf[:, PC * P : PC * P + PC],
                    start=True, stop=True,
                )
                sq_row = misc.tile([1, PC], F32, tag="m1")
                nc.scalar.activation(out=sq_row, in_=prow, func=AF.Sqrt)
                rq_row = misc.tile([1, PC], F32, tag="m2")
                nc.vector.reciprocal(rq_row, sq_row)
                nc.vector.tensor_tensor(
                    rq_row, rq_row, temprow[:, PC * P : PC * P + PC], op=OP.mult
                )
                rq_bf = misc.tile([1, PC], BF16, tag="m3")
                nc.vector.tensor_copy(rq_bf, rq_row)
                # rk as a column [PC, 1]
                pcol = ps.tile([PC, 1], F32, tag="ps")
                nc.tensor.matmul(
                    pcol, partials_bf[:, C + PC * P : C + PC * P + PC], onescol,
                    start=True, stop=True,
                )
                sq_col = misc.tile([PC, 1], F32, tag="m4")
                nc.scalar.activation(out=sq_col, in_=pcol, func=AF.Sqrt)
                rk_col = misc.tile([PC, 1], F32, tag="m5")
                nc.vector.reciprocal(rk_col, sq_col)
                # rq replicated across partitions via K=1 matmul
                prep = ps.tile([PC, PC], F32, tag="ps")
                nc.tensor.matmul(
                    prep, ones1[:, 0:PC], rq_bf, start=True, stop=True
                )
                rqrep = misc.tile([PC, PC], F32, tag="m6")
                nc.vector.tensor_copy(rqrep, prep)
                t1 = misc.tile([PC, PC], F32, tag="m7")
                nc.vector.tensor_tensor(t1, attnp[0:PC, :], rqrep, op=OP.mult)
                # exp(rk * t1), then zero junk blocks, bf16
                e1 = misc.tile([PC, PC], F32, tag="m8")
                nc.scalar.activation(out=e1, in_=t1, func=AF.Exp, scale=rk_col)
                ezero = stats.tile([PC, 128], BF16, tag=f"ez{P}")
                nc.vector.memset(ezero[:, PC:128], 0.0)
                nc.vector.tensor_tensor(ezero[:, 0:PC], e1, maskbd, op=OP.mult)
                # column sums -> recip
                pcs = ps.tile([PC, 1], F32, tag="ps")
                nc.tensor.matmul(
                    pcs, ezero[:, 0:PC], onescol[0:PC], start=True, stop=True
                )
                recip = stats.tile([PC, 1], F32, tag=f"rc{P}")
                nc.vector.reciprocal(recip, pcs)
                ezs.append((ezero, recip))

            # ================= Phase E: v depthwise =========================
            dw_phase(bv3, tkv_d, 7, C)

            # ================= Phase F: transpose v -> [c, hw] ==============
            vt = s1.tile([PC, 2 * HW], BF16, tag="qv")
            for P in range(2):
                for h in range(Himg):
                    ptv = pst.tile([PC, 128], BF16, tag="tp")
                    nc.tensor.transpose(
                        ptv, bv[:, h * C + PC * P : h * C + PC * P + PC], identb
                    )
                    nc.vector.tensor_copy(
                        vt[:, P * HW + h * 128 : P * HW + (h + 1) * 128], ptv
                    )

            # ================= Phase G: fused (attn @ v) + proj =============
            mps = []
            for P in range(2):
                ezero, recip = ezs[P]
                ezt_ps = pst.tile([PC, PC], BF16, tag="tp")
                nc.tensor.transpose(ezt_ps, ezero[:, 0:PC], identb[0:PC, 0:PC])
                ezt = misc.tile([PC, PC], BF16, tag="m9")
                nc.vector.tensor_copy(ezt, ezt_ps)
                wsc = misc.tile([PC, C], BF16, tag="m10")
                nc.vector.tensor_scalar_mul(wsc, (wp0, wp1)[P], recip)
                pmp = ps.tile([PC, C], F32, tag="ps")
                nc.tensor.matmul(pmp, ezt, wsc, start=True, stop=True)
                mp = stats.tile([PC, C], BF16, tag=f"mp{P}")
                nc.vector.tensor_copy(mp, pmp)
                mps.append(mp)

            for mi, (r0, r1) in enumerate(((0, 128), (128, 192))):
                mw = r1 - r0
                for n in range(0, HW, 512):
                    po = ps.tile([mw, 512], F32, tag="ps")
                    nc.tensor.matmul(
                        po, mps[0][:, r0:r1], vt[:, n : n + 512],
                        start=True, stop=False,
                    )
                    nc.tensor.matmul(
                        po, mps[1][:, r0:r1], vt[:, HW + n : HW + n + 512],
                        start=False, stop=True,
                    )
                    so = ostage.tile([mw, 512], F32, tag="os")
                    nc.vector.tensor_copy(so, po)
                    nc.sync.dma_start(out=out_d[r0:r1, n : n + 512], in_=so)

    _split_excess_waits(nc)
    return nc


def _get_program():
    global _PROG
    if _PROG is None:
        _PROG = _build_program()
    return _PROG


def kernel(x, y, q_w, q_dw_w, kv_w, kv_dw_w, proj_w, temperature):
    return _run(x, y, q_w, q_dw_w, kv_w, kv_dw_w, proj_w, temperature)[0]


def _run(x, y, q_w, q_dw_w, kv_w, kv_dw_w, proj_w, temperature, trace=False):
    from concourse.bass_utils import run_bass_kernel_spmd

    x = np.asarray(x, dtype=np.float32).reshape(B, C, HW).astype(ml_dtypes.bfloat16)
    y = np.asarray(y, dtype=np.float32).reshape(B, C, HW).astype(ml_dtypes.bfloat16)
    q_w = np.asarray(q_w, dtype=np.float32)
    kv_w = np.asarray(kv_w, dtype=np.float32)
    proj_w = np.asarray(proj_w, dtype=np.float32)
    q_dw_w = np.asarray(q_dw_w, dtype=np.float32)
    kv_dw_w = np.asarray(kv_dw_w, dtype=np.float32)
    temperature = np.asarray(temperature, dtype=np.float32).reshape(HEADS)

    wq = np.ascontiguousarray(q_w[:, :, 0, 0].T.astype(ml_dtypes.bfloat16))
    wkv = np.ascontiguousarray(kv_w[:, :, 0, 0].T.astype(ml_dtypes.bfloat16))  # [C, 2C]
    wpT = proj_w[:, :, 0, 0].T                              # [c_in, c_out]
    wp = np.stack([wpT[0:PC], wpT[PC:C]]).astype(ml_dtypes.bfloat16)
    tq = _build_toeplitz(q_dw_w[:, 0], 3)
    tkv = _build_toeplitz(kv_dw_w[:, 0], 7)
    idb = np.eye(128, dtype=ml_dtypes.bfloat16)
    maskbd = np.zeros((PC, PC), np.float32)
    maskbd[0:DHC, 0:DHC] = 1.0
    maskbd[DHC:PC, DHC:PC] = 1.0
    temprow = np.repeat(temperature, DHC).reshape(1, C)

    shared = {
        "wq": wq, "wkv": wkv, "wp": wp, "tq": tq, "tkv": tkv,
        "idb": idb, "maskbd": maskbd, "temprow": temprow,
    }
    in_maps = [dict(shared, x=x[i], y=y[i]) for i in range(B)]

    nc = _get_program()
    res = run_bass_kernel_spmd(
        nc, in_maps, core_ids=list(range(B)), trace=trace
    )
    out = np.stack([res.results[i]["out"] for i in range(B)])
    return out.reshape(B, C, Himg, Wimg).astype(np.float32), res



# revision 14
# speedup vs baseline: 1.9677x; 1.6451x over previous
"""nn_CrossAttention Trainium2 kernel — 8-core data-parallel over batch.

Per core (batch slice b=1):
  q1^T / kv1^T via transposed-orientation 1x1 convs (stationary = input rows,
  f32r matmuls), depthwise 3x3/7x7 as per-(channel, dh) banded-Toeplitz
  matmuls on the TensorEngine (host-built bf16 Toeplitz tiles, PSUM
  accumulation over dh with free-dim h shifts), l2-norm folded into attention
  scales, QK^T with n on partitions, softmax without max-subtraction
  (|logits| <= temperature), attn@v merged with the output 1x1 conv through
  a per-pair [96,192] fused matrix.
"""

import sys

sys.path.insert(0, "/opt/trn_rl_repo")

import numpy as np
import ml_dtypes

B, C, Himg, Wimg = 8, 192, 128, 128
HW = Himg * Wimg
HEADS, DHC = 4, 48      # heads, channels per head
PC = 96                 # channels per head-pair
NW = 8                  # Toeplitz tiles per DMA wave
SLAB = 8                # image rows per input stream slab

_PROG = None            # cached (nc, meta)


def _build_toeplitz(wdw, ksz):
    """wdw [c, ksz, ksz] f32 -> [128, c*ksz, 128] bf16, tile index = c*ksz + dh.

    T[w_in, tile, w_out] = wdw[c, dh, w_in - w_out + pad] inside the band,
    else 0.  Partition-major so a wave DMA reads contiguous bytes per
    partition.
    """
    pad = ksz // 2
    wi = np.arange(128)[:, None]
    wo = np.arange(128)[None, :]
    idx = wi - wo + pad
    valid = (idx >= 0) & (idx < ksz)
    idxc = np.clip(idx, 0, ksz - 1)
    T = wdw[:, :, idxc] * valid[None, None]          # [c, ksz, 128, 128]
    T = T.reshape(-1, 128, 128).transpose(1, 0, 2)   # [128, c*ksz, 128]
    return np.ascontiguousarray(T.astype(ml_dtypes.bfloat16))


def _split_excess_waits(nc, limit=1):
    """This container's walrus rejects >1 sync wait per instruction (and any
    wait on Drain beyond its own barrier). Hoist extras onto same-engine
    NoOps placed immediately before."""
    import bass_rust
    import concourse.mybir as mybir

    n_split = 0
    for fn in nc.m.functions:
        for bb in fn.blocks:
            insts = bb.instructions
            i = 0
            while i < len(insts):
                inst = insts[i]
                si = inst.sync_info
                lim = 0 if type(inst).__name__ == "InstDrain" else limit
                if si is not None and si.on_wait and len(si.on_wait) > lim:
                    waits = list(si.on_wait)
                    keep, extra = waits[:lim], waits[lim:]
                    pos = i
                    for j in range(0, len(extra), max(limit, 1)):
                        ch = extra[j : j + max(limit, 1)]
                        nop = mybir.InstNoOp(
                            name=f"waitsplit_{n_split}_{pos}",
                            engine=inst.engine,
                            ins=[],
                            outs=[],
                            sync_info=bass_rust.SyncInfo(on_wait=ch, on_update=[]),
                        )
                        insts.insert(pos, nop)
                        pos += 1
                        n_split += 1
                    inst.sync_info = bass_rust.SyncInfo(
                        on_wait=keep, on_update=list(si.on_update)
                    )
                    i = pos + 1
                else:
                    i += 1
    return n_split


def _build_program():
    import concourse.bass as bass
    import concourse.mybir as mybir
    import concourse.tile as tile

    F32 = mybir.dt.float32
    F32R = mybir.dt.float32r
    BF16 = mybir.dt.bfloat16
    AF = mybir.ActivationFunctionType
    OP = mybir.AluOpType

    nc = bass.Bass("TRN2", target_bir_lowering=False, debug=False, num_devices=8)

    # ---- DRAM parameters ----
    xin = nc.dram_tensor("x", [C, HW], BF16, kind="ExternalInput").ap()
    yin = nc.dram_tensor("y", [C, HW], BF16, kind="ExternalInput").ap()
    wq_d = nc.dram_tensor("wq", [C, C], BF16, kind="ExternalInput").ap()
    wkv_d = nc.dram_tensor("wkv", [C, 384], BF16, kind="ExternalInput").ap()
    wp_d = nc.dram_tensor("wp", [2, PC, C], BF16, kind="ExternalInput").ap()
    tq_d = nc.dram_tensor("tq", [128, C * 3, 128], BF16, kind="ExternalInput").ap()
    tkv_d = nc.dram_tensor("tkv", [128, 2 * C * 7, 128], BF16, kind="ExternalInput").ap()
    idb_d = nc.dram_tensor("idb", [128, 128], BF16, kind="ExternalInput").ap()
    mask_d = nc.dram_tensor("maskbd", [PC, PC], F32, kind="ExternalInput").ap()
    temp_d = nc.dram_tensor("temprow", [1, C], F32, kind="ExternalInput").ap()
    out_d = nc.dram_tensor("out", [C, HW], F32, kind="ExternalOutput").ap()

    with tile.TileContext(nc) as tc:
        import contextlib

        with contextlib.ExitStack() as ctx:
            consts = ctx.enter_context(tc.tile_pool(name="consts", bufs=1))
            s1 = ctx.enter_context(tc.tile_pool(name="s1", bufs=1))
            s2 = ctx.enter_context(tc.tile_pool(name="s2", bufs=1))
            s3 = ctx.enter_context(tc.tile_pool(name="s3", bufs=1))
            streams = ctx.enter_context(tc.tile_pool(name="streams", bufs=2))
            tpool = ctx.enter_context(tc.tile_pool(name="tpool", bufs=3))
            ps = ctx.enter_context(tc.tile_pool(name="ps", bufs=4, space="PSUM"))
            pst = ctx.enter_context(tc.tile_pool(name="pst", bufs=2, space="PSUM"))
            scratch = ctx.enter_context(tc.tile_pool(name="scratch", bufs=2))
            ostage = ctx.enter_context(tc.tile_pool(name="ostage", bufs=2))
            misc = ctx.enter_context(tc.tile_pool(name="misc", bufs=4))
            attnp_pool = ctx.enter_context(tc.tile_pool(name="attnp", bufs=2, space="PSUM"))
            stats = ctx.enter_context(tc.tile_pool(name="stats", bufs=1))

            # ---- load constants ----
            wq0 = consts.tile([128, C], BF16)
            wq1 = consts.tile([64, C], BF16)
            nc.sync.dma_start(out=wq0, in_=wq_d[0:128, :])
            nc.sync.dma_start(out=wq1, in_=wq_d[128:192, :])
            wkv0 = consts.tile([128, 384], BF16)
            wkv1 = consts.tile([64, 384], BF16)
            nc.sync.dma_start(out=wkv0, in_=wkv_d[0:128, :])
            nc.sync.dma_start(out=wkv1, in_=wkv_d[128:192, :])
            wp0 = consts.tile([PC, C], BF16)
            wp1 = consts.tile([PC, C], BF16)
            nc.sync.dma_start(out=wp0, in_=wp_d[0])
            nc.sync.dma_start(out=wp1, in_=wp_d[1])
            identb = consts.tile([128, 128], BF16)
            nc.sync.dma_start(out=identb, in_=idb_d)
            maskbd = consts.tile([PC, PC], F32)
            nc.sync.dma_start(out=maskbd, in_=mask_d)
            temprow = consts.tile([1, C], F32)
            nc.sync.dma_start(out=temprow, in_=temp_d)
            onescol = consts.tile([128, 1], BF16)
            nc.vector.memset(onescol, 1.0)
            ones1 = consts.tile([1, 128], BF16)
            nc.vector.memset(ones1, 1.0)

            # ---- big SBUF regions ----
            # channel-major: [w partitions, c*128 + h]; dw moving is contiguous
            bq = s1.tile([128, Himg * C], BF16, tag="qv")
            bk = s2.tile([128, Himg * C + 32], BF16, tag="kk")
            bv = s3.tile([128, Himg * C], BF16, tag="vv")
            bq3 = bq.rearrange("p (c h) -> p c h", h=Himg)
            bk3 = bk[:, 0 : Himg * C].rearrange("p (c h) -> p c h", h=Himg)
            bv3 = bv.rearrange("p (c h) -> p c h", h=Himg)
            # transposed views: [w, h, c] (strided in c)
            bq_hc = bq.rearrange("p (c h) -> p h c", h=Himg)
            bk_hc = bk[:, 0 : Himg * C].rearrange("p (c h) -> p h c", h=Himg)
            bv_hc = bv.rearrange("p (c h) -> p h c", h=Himg)

            partials = stats.tile([128, 2 * C], F32)
            partials_bf = stats.tile([128, 2 * C], BF16)

            def chan_ap(region3, c, col0, cnt):
                # [128, cnt] contiguous view: channel c, h col0..col0+cnt
                return region3[:, c, col0 : col0 + cnt]

            # ================= Phase A: 1x1 convs (transposed orientation) ==
            def conv1x1_phase(src_d, mov0, mov1, nmov, h2, wb):
                pt = [None]
                for h in range(Himg):
                    sl = h % SLAB
                    if sl == 0:
                        xs0 = streams.tile([128, SLAB * 128], BF16, tag="st0")
                        xs1 = streams.tile([64, SLAB * 128], BF16, tag="st1")
                        nc.sync.dma_start(
                            out=xs0, in_=src_d[0:128, h * 128 : (h + SLAB) * 128]
                        )
                        nc.sync.dma_start(
                            out=xs1, in_=src_d[128:192, h * 128 : (h + SLAB) * 128]
                        )
                    if h % h2 == 0:
                        pt[0] = ps.tile([128, h2 * nmov], F32, tag="ps", name=f"pt_{nmov}_{h}")
                    off = (h % h2) * nmov
                    nc.tensor.matmul(
                        pt[0][:, off : off + nmov],
                        xs0[:, sl * 128 : (sl + 1) * 128], mov0,
                        start=True, stop=False,
                    )
                    nc.tensor.matmul(
                        pt[0][:, off : off + nmov],
                        xs1[:, sl * 128 : (sl + 1) * 128], mov1,
                        start=False, stop=True,
                    )
                    if h % h2 == h2 - 1:
                        wb(h - h2 + 1, pt[0])

            def copy_on(eng_idx, dst, src):
                if eng_idx == 0:
                    nc.vector.tensor_copy(dst, src)
                else:
                    nc.scalar.activation(out=dst, in_=src, func=AF.Copy)

            def wb_q(h0, pt):
                # pt [128, 2*C] = (h, c); dst view [p, h, c]
                copy_on((h0 // 2) % 2, bq_hc[:, h0 : h0 + 2, :], pt)

            def wb_kv(h0, pt):
                copy_on(h0 % 2, bk_hc[:, h0, :], pt[:, 0:C])
                copy_on((h0 + 1) % 2, bv_hc[:, h0, :], pt[:, C : 2 * C])

            conv1x1_phase(xin, wq0, wq1, C, 2, wb_q)
            conv1x1_phase(yin, wkv0, wkv1, 384, 1, wb_kv)

            # ================= Phase B: depthwise via Toeplitz matmuls ======
            GB = 4                      # channels per PSUM bank group

            def dw_phase(region3, t_dram, ksz, ch0, sq_off=None):
                pad = ksz // 2
                order = [pad] + [d for d in range(ksz) if d != pad]
                cw = 4 if ksz == 3 else 2   # channels per T-wave
                wave_tile = [None]
                pdw4 = [None]
                for ci in range(C):
                    if ci % cw == 0:
                        nt = min(cw, C - ci) * ksz
                        wave_tile[0] = tpool.tile([128, cw * ksz, 128], BF16, tag="tw", name=f"tw_{ch0}_{ci}")
                        i0 = (ch0 + ci) * ksz
                        nc.sync.dma_start(
                            out=wave_tile[0][:, 0:nt, :],
                            in_=t_dram[:, i0 : i0 + nt, :],
                        )
                    tw = wave_tile[0]
                    if ci % GB == 0:
                        pdw4[0] = ps.tile([128, GB * 128], F32, tag="ps", name=f"pdw_{ch0}_{ci}")
                    base = (ci % cw) * ksz
                    slot = (ci % GB) * 128
                    for j, dh in enumerate(order):
                        sh = dh - pad
                        cnt = Himg - abs(sh)
                        h0o, h0i = max(0, -sh), max(0, sh)
                        nc.tensor.matmul(
                            pdw4[0][:, slot + h0o : slot + h0o + cnt],
                            tw[:, base + dh, :],
                            chan_ap(region3, ci, h0i, cnt),
                            start=(j == 0),
                            stop=(j == len(order) - 1),
                        )
                    if ci % GB == GB - 1:
                        g0 = ci - (GB - 1)
                        # group writeback: DVE for q/k (ACT busy with squares),
                        # alternate for v
                        eng = (ci // GB) % 2 if sq_off is None else 0
                        copy_on(eng, region3[:, g0 : ci + 1, :], pdw4[0])
                        if sq_off is not None:
                            for cc in range(g0, ci + 1):
                                sc = scratch.tile([128, 128], BF16, tag="sq")
                                nc.scalar.activation(
                                    out=sc,
                                    in_=pdw4[0][:, (cc - g0) * 128 : (cc - g0 + 1) * 128],
                                    func=AF.Square,
                                    accum_out=partials[:, sq_off + cc : sq_off + cc + 1],
                                )

            dw_phase(bq3, tq_d, 3, 0, sq_off=0)
            dw_phase(bk3, tkv_d, 7, 0, sq_off=C)
            nc.vector.tensor_copy(partials_bf, partials)

            # ================= Phase D: QK^T + softmax prep per pair ========
            ezs = []
            for P in range(2):
                attnp = attnp_pool.tile([PC, PC], F32, tag="at")
                for h in range(Himg):
                    nc.tensor.matmul(
                        attnp,
                        bk_hc[:, h, PC * P : PC * P + PC],
                        bq_hc[:, h, PC * P : PC * P + PC],
                        start=(h == 0),
                        stop=(h == Himg - 1),
                    )
                # rq as a row [1, PC]: colsum of q-partials then rsqrt, * temp
                prow = ps.tile([1, PC], F32, tag="ps")
                nc.tensor.matmul(
                    prow, onescol, partials_bf[:, PC * P : PC * P + PC],
                    start=True, stop=True,
                )
                sq_row = misc.tile([1, PC], F32, tag="m1")
                nc.scalar.activation(out=sq_row, in_=prow, func=AF.Sqrt)
                rq_row = misc.tile([1, PC], F32, tag="m2")
                nc.vector.reciprocal(rq_row, sq_row)
                nc.vector.tensor_tensor(
                    rq_row, rq_row, temprow[:, PC * P : PC * P + PC], op=OP.mult
                )
                rq_bf = misc.tile([1, PC], BF16, tag="m3")
                nc.vector.tensor_copy(rq_bf, rq_row)
                # rk as a column [PC, 1]
                pcol = ps.tile([PC, 1], F32, tag="ps")
                nc.tensor.matmul(
                    pcol, partials_bf[:, C + PC * P : C + PC * P + PC], onescol,
                    start=True, stop=True,
                )
                sq_col = misc.tile([PC, 1], F32, tag="m4")
                nc.scalar.activation(out=sq_col, in_=pcol, func=AF.Sqrt)
                rk_col = misc.tile([PC, 1], F32, tag="m5")
                nc.vector.reciprocal(rk_col, sq_col)
                # rq replicated across partitions via K=1 matmul
                prep = ps.tile([PC, PC], F32, tag="ps")
                nc.tensor.matmul(
                    prep, ones1[:, 0:PC], rq_bf, start=True, stop=True
                )
                rqrep = misc.tile([PC, PC], F32, tag="m6")
                nc.vector.tensor_copy(rqrep, prep)
                t1 = misc.tile([PC, PC], F32, tag="m7")
                nc.vector.tensor_tensor(t1, attnp, rqrep, op=OP.mult)
                # exp(rk * t1), then zero junk blocks, bf16
                e1 = misc.tile([PC, PC], F32, tag="m8")
                nc.scalar.activation(out=e1, in_=t1, func=AF.Exp, scale=rk_col)
                ezero = stats.tile([PC, 128], BF16, tag=f"ez{P}")
                nc.vector.memset(ezero[:, PC:128], 0.0)
                nc.vector.tensor_tensor(ezero[:, 0:PC], e1, maskbd, op=OP.mult)
                # column sums -> recip
                pcs = ps.tile([PC, 1], F32, tag="ps")
                nc.tensor.matmul(
                    pcs, ezero[:, 0:PC], onescol[0:PC], start=True, stop=True
                )
                recip = stats.tile([PC, 1], F32, tag=f"rc{P}")
                nc.vector.reciprocal(recip, pcs)
                ezs.append((ezero, recip))

            # ================= Phase E: v depthwise =========================
            dw_phase(bv3, tkv_d, 7, C)

            # ================= Phase F: transpose v -> [c, hw] ==============
            vt = s1.tile([PC, 2 * HW], BF16, tag="qv")
            for P in range(2):
                for h in range(Himg):
                    ptv = pst.tile([PC, 128], BF16, tag="tp")
                    nc.tensor.transpose(
                        ptv, bv_hc[:, h, PC * P : PC * P + PC], identb
                    )
                    nc.vector.tensor_copy(
                        vt[:, P * HW + h * 128 : P * HW + (h + 1) * 128], ptv
                    )

            # ================= Phase G: fused (attn @ v) + proj =============
            mps = []
            for P in range(2):
                ezero, recip = ezs[P]
                ezt_ps = pst.tile([PC, PC], BF16, tag="tp")
                nc.tensor.transpose(ezt_ps, ezero[:, 0:PC], identb[0:PC, 0:PC])
                ezt = misc.tile([PC, PC], BF16, tag="m9")
                nc.vector.tensor_copy(ezt, ezt_ps)
                wsc = misc.tile([PC, C], BF16, tag="m10")
                nc.vector.tensor_scalar_mul(wsc, (wp0, wp1)[P], recip)
                pmp = ps.tile([PC, C], F32, tag="ps")
                nc.tensor.matmul(pmp, ezt, wsc, start=True, stop=True)
                mp = stats.tile([PC, C], BF16, tag=f"mp{P}")
                nc.vector.tensor_copy(mp, pmp)
                mps.append(mp)

            for mi, (r0, r1) in enumerate(((0, 128), (128, 192))):
                mw = r1 - r0
                for n in range(0, HW, 512):
                    po = ps.tile([mw, 512], F32, tag="ps")
                    nc.tensor.matmul(
                        po, mps[0][:, r0:r1], vt[:, n : n + 512],
                        start=True, stop=False,
                    )
                    nc.tensor.matmul(
                        po, mps[1][:, r0:r1], vt[:, HW + n : HW + n + 512],
                        start=False, stop=True,
                    )
                    so = ostage.tile([mw, 512], F32, tag="os")
                    nc.vector.tensor_copy(so, po)
                    nc.sync.dma_start(out=out_d[r0:r1, n : n + 512], in_=so)

    _split_excess_waits(nc)
    return nc


def _get_program():
    global _PROG
    if _PROG is None:
        _PROG = _build_program()
    return _PROG


def kernel(x, y, q_w, q_dw_w, kv_w, kv_dw_w, proj_w, temperature):
    return _run(x, y, q_w, q_dw_w, kv_w, kv_dw_w, proj_w, temperature)[0]


def _run(x, y, q_w, q_dw_w, kv_w, kv_dw_w, proj_w, temperature, trace=False):
    from concourse.bass_utils import run_bass_kernel_spmd

    x = np.asarray(x, dtype=np.float32).reshape(B, C, HW).astype(ml_dtypes.bfloat16)
    y = np.asarray(y, dtype=np.float32).reshape(B, C, HW).astype(ml_dtypes.bfloat16)
    q_w = np.asarray(q_w, dtype=np.float32)
    kv_w = np.asarray(kv_w, dtype=np.float32)
    proj_w = np.asarray(proj_w, dtype=np.float32)
    q_dw_w = np.asarray(q_dw_w, dtype=np.float32)
    kv_dw_w = np.asarray(kv_dw_w, dtype=np.float32)
    temperature = np.asarray(temperature, dtype=np.float32).reshape(HEADS)

    wq = np.ascontiguousarray(q_w[:, :, 0, 0].T.astype(ml_dtypes.bfloat16))
    wkv = np.ascontiguousarray(kv_w[:, :, 0, 0].T.astype(ml_dtypes.bfloat16))  # [C, 2C]
    wpT = proj_w[:, :, 0, 0].T                              # [c_in, c_out]
    wp = np.stack([wpT[0:PC], wpT[PC:C]]).astype(ml_dtypes.bfloat16)
    tq = _build_toeplitz(q_dw_w[:, 0], 3)
    tkv = _build_toeplitz(kv_dw_w[:, 0], 7)
    idb = np.eye(128, dtype=ml_dtypes.bfloat16)
    maskbd = np.zeros((PC, PC), np.float32)
    maskbd[0:DHC, 0:DHC] = 1.0
    maskbd[DHC:PC, DHC:PC] = 1.0
    temprow = np.repeat(temperature, DHC).reshape(1, C)

    shared = {
        "wq": wq, "wkv": wkv, "wp": wp, "tq": tq, "tkv": tkv,
        "idb": idb, "maskbd": maskbd, "temprow": temprow,
    }
    in_maps = [dict(shared, x=x[i], y=y[i]) for i in range(B)]

    nc = _get_program()
    res = run_bass_kernel_spmd(
        nc, in_maps, core_ids=list(range(B)), trace=trace
    )
    out = np.stack([res.results[i]["out"] for i in range(B)])
    return out.reshape(B, C, Himg, Wimg).astype(np.float32), res



# revision 20
# speedup vs baseline: 2.0695x; 1.0517x over previous
"""nn_CrossAttention Trainium2 kernel — 8-core data-parallel over batch.

Per core (batch slice b=1):
  q1^T / kv1^T via transposed-orientation 1x1 convs (stationary = input rows,
  f32r matmuls), depthwise 3x3/7x7 as per-(channel, dh) banded-Toeplitz
  matmuls on the TensorEngine (host-built bf16 Toeplitz tiles, PSUM
  accumulation over dh with free-dim h shifts), l2-norm folded into attention
  scales, QK^T with n on partitions, softmax without max-subtraction
  (|logits| <= temperature), attn@v merged with the output 1x1 conv through
  a per-pair [96,192] fused matrix.
"""

import sys

sys.path.insert(0, "/opt/trn_rl_repo")

import numpy as np
import ml_dtypes

B, C, Himg, Wimg = 8, 192, 128, 128
HW = Himg * Wimg
HEADS, DHC = 4, 48      # heads, channels per head
PC = 96                 # channels per head-pair
NW = 8                  # Toeplitz tiles per DMA wave
SLAB = 4                # image rows per input stream slab

_PROG = None            # cached (nc, meta)


def _build_toeplitz(wdw, ksz):
    """wdw [c, ksz, ksz] f32 -> [128, c*ksz, 128] bf16, tile index = c*ksz + dh.

    T[w_in, tile, w_out] = wdw[c, dh, w_in - w_out + pad] inside the band,
    else 0.  Partition-major so a wave DMA reads contiguous bytes per
    partition.
    """
    pad = ksz // 2
    wi = np.arange(128)[:, None]
    wo = np.arange(128)[None, :]
    idx = wi - wo + pad
    valid = (idx >= 0) & (idx < ksz)
    idxc = np.clip(idx, 0, ksz - 1)
    T = wdw[:, :, idxc] * valid[None, None]          # [c, ksz, 128, 128]
    T = T.reshape(-1, 128, 128).transpose(1, 0, 2)   # [128, c*ksz, 128]
    return np.ascontiguousarray(T.astype(ml_dtypes.bfloat16))


def _split_excess_waits(nc, limit=1):
    """This container's walrus rejects >1 sync wait per instruction (and any
    wait on Drain beyond its own barrier). Hoist extras onto same-engine
    NoOps placed immediately before."""
    import bass_rust
    import concourse.mybir as mybir

    n_split = 0
    for fn in nc.m.functions:
        for bb in fn.blocks:
            insts = bb.instructions
            i = 0
            while i < len(insts):
                inst = insts[i]
                si = inst.sync_info
                lim = 0 if type(inst).__name__ == "InstDrain" else limit
                if si is not None and si.on_wait and len(si.on_wait) > lim:
                    waits = list(si.on_wait)
                    keep, extra = waits[:lim], waits[lim:]
                    pos = i
                    for j in range(0, len(extra), max(limit, 1)):
                        ch = extra[j : j + max(limit, 1)]
                        nop = mybir.InstNoOp(
                            name=f"waitsplit_{n_split}_{pos}",
                            engine=inst.engine,
                            ins=[],
                            outs=[],
                            sync_info=bass_rust.SyncInfo(on_wait=ch, on_update=[]),
                        )
                        insts.insert(pos, nop)
                        pos += 1
                        n_split += 1
                    inst.sync_info = bass_rust.SyncInfo(
                        on_wait=keep, on_update=list(si.on_update)
                    )
                    i = pos + 1
                else:
                    i += 1
    return n_split


def _build_program():
    import concourse.bass as bass
    import concourse.mybir as mybir
    import concourse.tile as tile

    F32 = mybir.dt.float32
    F32R = mybir.dt.float32r
    BF16 = mybir.dt.bfloat16
    AF = mybir.ActivationFunctionType
    OP = mybir.AluOpType

    nc = bass.Bass("TRN2", target_bir_lowering=False, debug=False, num_devices=8)

    # ---- DRAM parameters ----
    xin = nc.dram_tensor("x", [C, HW], BF16, kind="ExternalInput").ap()
    yin = nc.dram_tensor("y", [C, HW], BF16, kind="ExternalInput").ap()
    wq_d = nc.dram_tensor("wq", [C, C], BF16, kind="ExternalInput").ap()
    wkv_d = nc.dram_tensor("wkv", [C, 384], BF16, kind="ExternalInput").ap()
    wp_d = nc.dram_tensor("wp", [2, PC, C], BF16, kind="ExternalInput").ap()
    tq_d = nc.dram_tensor("tq", [128, C * 3, 128], BF16, kind="ExternalInput").ap()
    tkv_d = nc.dram_tensor("tkv", [128, 2 * C * 7, 128], BF16, kind="ExternalInput").ap()
    idb_d = nc.dram_tensor("idb", [128, 128], BF16, kind="ExternalInput").ap()
    mask_d = nc.dram_tensor("maskbd", [PC, PC], F32, kind="ExternalInput").ap()
    temp_d = nc.dram_tensor("temprow", [1, C], F32, kind="ExternalInput").ap()
    out_d = nc.dram_tensor("out", [C, HW], F32, kind="ExternalOutput").ap()

    with tile.TileContext(nc) as tc:
        import contextlib

        with contextlib.ExitStack() as ctx:
            consts = ctx.enter_context(tc.tile_pool(name="consts", bufs=1))
            s1 = ctx.enter_context(tc.tile_pool(name="s1", bufs=1))
            s2 = ctx.enter_context(tc.tile_pool(name="s2", bufs=1))
            s3 = ctx.enter_context(tc.tile_pool(name="s3", bufs=1))
            streams = ctx.enter_context(tc.tile_pool(name="streams", bufs=2))
            tpool = ctx.enter_context(tc.tile_pool(name="tpool", bufs=3))
            ps = ctx.enter_context(tc.tile_pool(name="ps", bufs=4, space="PSUM"))
            pst = ctx.enter_context(tc.tile_pool(name="pst", bufs=2, space="PSUM"))
            scratch = ctx.enter_context(tc.tile_pool(name="scratch", bufs=2))
            ostage = ctx.enter_context(tc.tile_pool(name="ostage", bufs=2))
            misc = ctx.enter_context(tc.tile_pool(name="misc", bufs=4))
            attnp_pool = ctx.enter_context(tc.tile_pool(name="attnp", bufs=2, space="PSUM"))
            stats = ctx.enter_context(tc.tile_pool(name="stats", bufs=1))

            # ---- load constants ----
            wq0 = consts.tile([128, C], BF16)
            wq1 = consts.tile([64, C], BF16)
            nc.sync.dma_start(out=wq0, in_=wq_d[0:128, :])
            nc.sync.dma_start(out=wq1, in_=wq_d[128:192, :])
            wkv0 = consts.tile([128, 384], BF16)
            wkv1 = consts.tile([64, 384], BF16)
            nc.sync.dma_start(out=wkv0, in_=wkv_d[0:128, :])
            nc.sync.dma_start(out=wkv1, in_=wkv_d[128:192, :])
            wp0 = consts.tile([PC, C], BF16)
            wp1 = consts.tile([PC, C], BF16)
            nc.sync.dma_start(out=wp0, in_=wp_d[0])
            nc.sync.dma_start(out=wp1, in_=wp_d[1])
            identb = consts.tile([128, 128], BF16)
            nc.sync.dma_start(out=identb, in_=idb_d)
            maskbd = consts.tile([PC, PC], F32)
            nc.sync.dma_start(out=maskbd, in_=mask_d)
            temprow = consts.tile([1, C], F32)
            nc.sync.dma_start(out=temprow, in_=temp_d)
            onescol = consts.tile([128, 1], BF16)
            nc.vector.memset(onescol, 1.0)
            ones1 = consts.tile([1, 128], BF16)
            nc.vector.memset(ones1, 1.0)

            # ---- big SBUF regions ----
            # channel-major: [w partitions, c*128 + h]; dw moving is contiguous
            bq = s1.tile([128, Himg * C], BF16, tag="qv")
            bk = s2.tile([128, Himg * C + 32], BF16, tag="kk")
            bv = s3.tile([128, Himg * C], BF16, tag="vv")
            bq3 = bq.rearrange("p (c h) -> p c h", h=Himg)
            bk3 = bk[:, 0 : Himg * C].rearrange("p (c h) -> p c h", h=Himg)
            bv3 = bv.rearrange("p (c h) -> p c h", h=Himg)
            # transposed views: [w, h, c] (strided in c)
            bq_hc = bq.rearrange("p (c h) -> p h c", h=Himg)
            bk_hc = bk[:, 0 : Himg * C].rearrange("p (c h) -> p h c", h=Himg)
            bv_hc = bv.rearrange("p (c h) -> p h c", h=Himg)

            # h-major staging ring for phase A (repacked to c-major by gpsimd)
            HSTG, CHUNK = 24, 8
            stg = stats.tile([128, HSTG * 384], BF16, name="stg")
            stg_hc = stg.rearrange("p (h c) -> p h c", c=384)
            stg_ch = stg.rearrange("p (h c) -> p c h", c=384)

            partials = stats.tile([128, 2 * C], F32)
            partials_bf = stats.tile([128, 2 * C], BF16)

            def chan_ap(region3, c, col0, cnt):
                # [128, cnt] contiguous view: channel c, h col0..col0+cnt
                return region3[:, c, col0 : col0 + cnt]

            def copy_on(eng_idx, dst, src):
                if eng_idx == 0:
                    nc.vector.tensor_copy(dst, src)
                else:
                    nc.scalar.activation(out=dst, in_=src, func=AF.Copy)

            # ================= Phase A: 1x1 convs (transposed orientation) ==
            def conv1x1_phase(src_d, mov0, mov1, nmov, h2, targets):
                pt = [None]
                for h in range(Himg):
                    sl = h % SLAB
                    if sl == 0:
                        xs0 = streams.tile([128, SLAB * 128], BF16, tag="st0")
                        xs1 = streams.tile([64, SLAB * 128], BF16, tag="st1")
                        nc.sync.dma_start(
                            out=xs0, in_=src_d[0:128, h * 128 : (h + SLAB) * 128]
                        )
                        nc.sync.dma_start(
                            out=xs1, in_=src_d[128:192, h * 128 : (h + SLAB) * 128]
                        )
                    if h % h2 == 0:
                        pt[0] = ps.tile([128, h2 * nmov], F32, tag="ps", name=f"pt_{nmov}_{h}")
                    off = (h % h2) * nmov
                    nc.tensor.matmul(
                        pt[0][:, off : off + nmov],
                        xs0[:, sl * 128 : (sl + 1) * 128], mov0,
                        start=True, stop=False,
                    )
                    nc.tensor.matmul(
                        pt[0][:, off : off + nmov],
                        xs1[:, sl * 128 : (sl + 1) * 128], mov1,
                        start=False, stop=True,
                    )
                    if h % h2 == h2 - 1:
                        h0 = h - h2 + 1
                        s0 = h0 % HSTG
                        # contiguous psum -> staging (alternating DVE/ACT)
                        copy_on(
                            (h0 // h2) % 2,
                            stg_hc[:, s0 : s0 + h2, 0:nmov],
                            pt[0],
                        )
                    if h % CHUNK == CHUNK - 1:
                        # gpsimd repack: h-major chunk -> c-major regions
                        hc0 = h - CHUNK + 1
                        s0 = hc0 % HSTG
                        for reg3, c0, c1 in targets:
                            nc.gpsimd.tensor_copy(
                                reg3[:, 0 : c1 - c0, hc0 : hc0 + CHUNK],
                                stg_ch[:, c0:c1, s0 : s0 + CHUNK],
                            )

            conv1x1_phase(xin, wq0, wq1, C, 2, [(bq3, 0, C)])
            conv1x1_phase(yin, wkv0, wkv1, 384, 1, [(bk3, 0, C), (bv3, C, 2 * C)])

            # ================= Phase B: depthwise via Toeplitz matmuls ======
            GB = 4                      # channels per PSUM bank group

            def dw_phase(region3, t_dram, ksz, ch0, sq_off=None):
                pad = ksz // 2
                order = [pad] + [d for d in range(ksz) if d != pad]
                cw = 4 if ksz == 3 else 2   # channels per T-wave
                wave_tile = [None]
                pdw4 = [None]
                for ci in range(C):
                    if ci % cw == 0:
                        nt = min(cw, C - ci) * ksz
                        wave_tile[0] = tpool.tile([128, cw * ksz, 128], BF16, tag="tw", name=f"tw_{ch0}_{ci}")
                        i0 = (ch0 + ci) * ksz
                        nc.sync.dma_start(
                            out=wave_tile[0][:, 0:nt, :],
                            in_=t_dram[:, i0 : i0 + nt, :],
                        )
                    tw = wave_tile[0]
                    if ci % GB == 0:
                        pdw4[0] = ps.tile([128, GB * 128], F32, tag="ps", name=f"pdw_{ch0}_{ci}")
                    base = (ci % cw) * ksz
                    slot = (ci % GB) * 128
                    for j, dh in enumerate(order):
                        sh = dh - pad
                        cnt = Himg - abs(sh)
                        h0o, h0i = max(0, -sh), max(0, sh)
                        nc.tensor.matmul(
                            pdw4[0][:, slot + h0o : slot + h0o + cnt],
                            tw[:, base + dh, :],
                            chan_ap(region3, ci, h0i, cnt),
                            start=(j == 0),
                            stop=(j == len(order) - 1),
                        )
                    if ci % GB == GB - 1:
                        g0 = ci - (GB - 1)
                        # group writeback: DVE for q/k (ACT busy with squares),
                        # alternate for v
                        eng = (ci // GB) % 2 if sq_off is None else 0
                        copy_on(eng, region3[:, g0 : ci + 1, :], pdw4[0])
                        if sq_off is not None:
                            # sum-of-squares from SBUF so the PSUM bank is
                            # released by the copy alone
                            for cc in range(g0, ci + 1):
                                sc = scratch.tile([128, 128], BF16, tag="sq")
                                nc.scalar.activation(
                                    out=sc,
                                    in_=region3[:, cc, :],
                                    func=AF.Square,
                                    accum_out=partials[:, sq_off + cc : sq_off + cc + 1],
                                )

            dw_phase(bq3, tq_d, 3, 0, sq_off=0)
            dw_phase(bk3, tkv_d, 7, 0, sq_off=C)
            nc.vector.tensor_copy(partials_bf, partials)

            # ================= Phase D: QK^T + softmax prep per pair ========
            ezs = []
            for P in range(2):
                attnp = attnp_pool.tile([PC, PC], F32, tag="at")
                for h in range(Himg):
                    nc.tensor.matmul(
                        attnp,
                        bk_hc[:, h, PC * P : PC * P + PC],
                        bq_hc[:, h, PC * P : PC * P + PC],
                        start=(h == 0),
                        stop=(h == Himg - 1),
                    )
                # rq as a row [1, PC]: colsum of q-partials then rsqrt, * temp
                prow = ps.tile([1, PC], F32, tag="ps")
                nc.tensor.matmul(
                    prow, onescol, partials_bf[:, PC * P : PC * P + PC],
                    start=True, stop=True,
                )
                sq_row = misc.tile([1, PC], F32, tag="m1")
                nc.scalar.activation(out=sq_row, in_=prow, func=AF.Sqrt)
                rq_row = misc.tile([1, PC], F32, tag="m2")
                nc.vector.reciprocal(rq_row, sq_row)
                nc.vector.tensor_tensor(
                    rq_row, rq_row, temprow[:, PC * P : PC * P + PC], op=OP.mult
                )
                rq_bf = misc.tile([1, PC], BF16, tag="m3")
                nc.vector.tensor_copy(rq_bf, rq_row)
                # rk as a column [PC, 1]
                pcol = ps.tile([PC, 1], F32, tag="ps")
                nc.tensor.matmul(
                    pcol, partials_bf[:, C + PC * P : C + PC * P + PC], onescol,
                    start=True, stop=True,
                )
                sq_col = misc.tile([PC, 1], F32, tag="m4")
                nc.scalar.activation(out=sq_col, in_=pcol, func=AF.Sqrt)
                rk_col = misc.tile([PC, 1], F32, tag="m5")
                nc.vector.reciprocal(rk_col, sq_col)
                # rq replicated across partitions via K=1 matmul
                prep = ps.tile([PC, PC], F32, tag="ps")
                nc.tensor.matmul(
                    prep, ones1[:, 0:PC], rq_bf, start=True, stop=True
                )
                rqrep = misc.tile([PC, PC], F32, tag="m6")
                nc.vector.tensor_copy(rqrep, prep)
                t1 = misc.tile([PC, PC], F32, tag="m7")
                nc.vector.tensor_tensor(t1, attnp, rqrep, op=OP.mult)
                # exp(rk * t1), then zero junk blocks, bf16
                e1 = misc.tile([PC, PC], F32, tag="m8")
                nc.scalar.activation(out=e1, in_=t1, func=AF.Exp, scale=rk_col)
                ezero = stats.tile([PC, 128], BF16, tag=f"ez{P}")
                nc.vector.memset(ezero[:, PC:128], 0.0)
                nc.vector.tensor_tensor(ezero[:, 0:PC], e1, maskbd, op=OP.mult)
                # column sums -> recip
                pcs = ps.tile([PC, 1], F32, tag="ps")
                nc.tensor.matmul(
                    pcs, ezero[:, 0:PC], onescol[0:PC], start=True, stop=True
                )
                recip = stats.tile([PC, 1], F32, tag=f"rc{P}")
                nc.vector.reciprocal(recip, pcs)
                ezs.append((ezero, recip))

            # ================= Phase E: v depthwise =========================
            dw_phase(bv3, tkv_d, 7, C)

            # ================= Phase G: fused (attn @ v) + proj =============
            mps = []
            for P in range(2):
                ezero, recip = ezs[P]
                ezt_ps = pst.tile([PC, PC], BF16, tag="tp")
                nc.tensor.transpose(ezt_ps, ezero[:, 0:PC], identb[0:PC, 0:PC])
                ezt = misc.tile([PC, PC], BF16, tag="m9")
                nc.vector.tensor_copy(ezt, ezt_ps)
                wsc = misc.tile([PC, C], BF16, tag="m10")
                nc.vector.tensor_scalar_mul(wsc, (wp0, wp1)[P], recip)
                pmp = ps.tile([PC, C], F32, tag="ps")
                nc.tensor.matmul(pmp, ezt, wsc, start=True, stop=True)
                mp = stats.tile([PC, C], BF16, tag=f"mp{P}")
                nc.vector.tensor_copy(mp, pmp)
                mps.append(mp)

            # per 512-col block: transpose 4 h-rows of v per pair (batched
            # into one PSUM tile), then the two fused output matmuls
            for nb in range(Himg // 4):
                h0 = nb * 4
                vtbs = []
                for P in range(2):
                    ptv = pst.tile([PC, 512], BF16, tag="tp", name=f"ptv{P}_{nb}")
                    for hh in range(4):
                        nc.tensor.transpose(
                            ptv[:, hh * 128 : (hh + 1) * 128],
                            bv_hc[:, h0 + hh, PC * P : PC * P + PC],
                            identb,
                        )
                    vtb = scratch.tile([PC, 512], BF16, tag=f"vtb{P}", name=f"vtb{P}_{nb}")
                    copy_on(P, vtb, ptv)
                    vtbs.append(vtb)
                n = nb * 512
                for mi, (r0, r1) in enumerate(((0, 128), (128, 192))):
                    mw = r1 - r0
                    po = ps.tile([mw, 512], F32, tag="ps", name=f"po_{mi}_{nb}")
                    nc.tensor.matmul(
                        po, mps[0][:, r0:r1], vtbs[0],
                        start=True, stop=False,
                    )
                    nc.tensor.matmul(
                        po, mps[1][:, r0:r1], vtbs[1],
                        start=False, stop=True,
                    )
                    so = ostage.tile([mw, 512], F32, tag="os", name=f"so_{mi}_{nb}")
                    copy_on(mi, so, po)
                    nc.sync.dma_start(out=out_d[r0:r1, n : n + 512], in_=so)

    _split_excess_waits(nc)
    return nc


def _get_program():
    global _PROG
    if _PROG is None:
        _PROG = _build_program()
    return _PROG


def kernel(x, y, q_w, q_dw_w, kv_w, kv_dw_w, proj_w, temperature):
    return _run(x, y, q_w, q_dw_w, kv_w, kv_dw_w, proj_w, temperature)[0]


def _run(x, y, q_w, q_dw_w, kv_w, kv_dw_w, proj_w, temperature, trace=False):
    from concourse.bass_utils import run_bass_kernel_spmd

    x = np.asarray(x, dtype=np.float32).reshape(B, C, HW).astype(ml_dtypes.bfloat16)
    y = np.asarray(y, dtype=np.float32).reshape(B, C, HW).astype(ml_dtypes.bfloat16)
    q_w = np.asarray(q_w, dtype=np.float32)
    kv_w = np.asarray(kv_w, dtype=np.float32)
    proj_w = np.asarray(proj_w, dtype=np.float32)
    q_dw_w = np.asarray(q_dw_w, dtype=np.float32)
    kv_dw_w = np.asarray(kv_dw_w, dtype=np.float32)
    temperature = np.asarray(temperature, dtype=np.float32).reshape(HEADS)

    wq = np.ascontiguousarray(q_w[:, :, 0, 0].T.astype(ml_dtypes.bfloat16))
    wkv = np.ascontiguousarray(kv_w[:, :, 0, 0].T.astype(ml_dtypes.bfloat16))  # [C, 2C]
    wpT = proj_w[:, :, 0, 0].T                              # [c_in, c_out]
    wp = np.stack([wpT[0:PC], wpT[PC:C]]).astype(ml_dtypes.bfloat16)
    tq = _build_toeplitz(q_dw_w[:, 0], 3)
    tkv = _build_toeplitz(kv_dw_w[:, 0], 7)
    idb = np.eye(128, dtype=ml_dtypes.bfloat16)
    maskbd = np.zeros((PC, PC), np.float32)
    maskbd[0:DHC, 0:DHC] = 1.0
    maskbd[DHC:PC, DHC:PC] = 1.0
    temprow = np.repeat(temperature, DHC).reshape(1, C)

    shared = {
        "wq": wq, "wkv": wkv, "wp": wp, "tq": tq, "tkv": tkv,
        "idb": idb, "maskbd": maskbd, "temprow": temprow,
    }
    in_maps = [dict(shared, x=x[i], y=y[i]) for i in range(B)]

    nc = _get_program()
    res = run_bass_kernel_spmd(
        nc, in_maps, core_ids=list(range(B)), trace=trace
    )
    out = np.stack([res.results[i]["out"] for i in range(B)])
    return out.reshape(B, C, Himg, Wimg).astype(np.float32), res

